# revision 1
# baseline (speedup 1.0000x reference)
"""Trainium2 Bass kernel for a FlowNet-style MPI correlation layer.

Reference computation (per batch b, shift s=(k,i,j), position p=(y,x,z)):
    cost[b,s,p]  = mean_c f1[b,c,p] * f2pad[b,c,p+delta_s]        (243 shifts)
    cmask[b,s,p] = clip(mask1[b,p] * m2pad[b,p+delta_s], 0, 1)
with mask1 = clip(sum_z alpha1, 0, 1) broadcast over z, f2 zero-padded,
m2 one-padded.

Strategy (8 NeuronCores, spatial shard over h: 12 rows/core):
  - Cost path: bf16 elementwise products (2x_1P perf mode), with c=64
    channels on the partition axis packed as (y-half, c) -> 128 partitions.
    Channel reduction runs on TensorE: ones-pattern weights reduce each
    64-row channel group, with 16 accumulating matmuls packing a full
    128x512 PSUM bank (32-row strips via tile_position).  ScalarE copies
    PSUM->SBUF; the 1/64 mean scale is folded into the bf16 cast of f1 on
    the host (exact: power of two).
  - The DVE is the bottleneck engine, so product formation is split
    between VectorE and the otherwise-idle Pool engine (GPSIMD, ~3.8x
    slower per element): 1-2 shifts per 7-shift PSUM group run on Pool,
    software-pipelined two groups ahead so TensorE never waits on the
    slower engine.  All mask-path multiplies also run on Pool.
  - dz=+1 shifts would mis-align bf16 pair-packed reads (odd element
    offset), so the host ships a second f2 copy pre-shifted by one z
    element; shifts are processed in dz-phases so one f2 halo buffer is
    live at a time and refills double-buffer.  Phase loads are prefetched
    well ahead on the sync (SP) HWDGE queue: their slot-WAR wait clears
    only when the outgoing phase finishes, and parking that wait on the
    ACT queue would gate the PSUM->SBUF copies and stall PE/DVE at every
    phase boundary.
  - dz=+1 shift muls skip their last z column (an exact zero against the
    f2 pad); the column is zeroed by same-queue memset primes on range
    entry, so PE reads genuine zeros ordered via engine FIFO + RAW sems.
  - Mask path: bf16 muls on Pool with two dx shifts packed as
    (dd, b, y) -> 96 partitions (the dd=1 half of the m2 halo is loaded
    pre-shifted by one x), halving Pool's per-element mask work; dy
    shifts are realized with small SBUF->SBUF partition-shift DMAs,
    dx/dz as free-dim offsets; ScalarE upcasts to fp32 before the store.
    One dx-pair is processed per cost group in program order.
"""

import numpy as np
import ml_dtypes
from contextlib import ExitStack

import concourse.bacc as bacc
import concourse.tile as tile
from concourse import mybir
from concourse.bass_utils import run_bass_kernel_spmd

# Problem shape (hardcoded per contest contract).
B, C, H, W, D = 4, 64, 96, 96, 8
S, SD = 4, 1                       # spatial / depth search range
NSX = 2 * S + 1                    # 9 shifts per spatial axis
NSD = 2 * SD + 1                   # 3 depth shifts
NS = NSX * NSX * NSD               # 243 total shifts
NSX2 = NSX * NSX
NCORES = 8
HS = H // NCORES                   # 12 rows of y per core
GH = HS // 2                       # 6 = y-half height (partition packing)
HP = HS + 2 * S                    # 20 = y rows incl halo
F2Y, F2X, F2Z = GH + 2 * S, W + 2 * S, D + 2 * SD   # 14, 104, 10
POS = GH * W * D                   # 4608 free positions per product tile
CHUNK = 512
NCHUNK = POS // CHUNK              # 9 matmul chunks per product tile
SGRP = 7                           # shifts per PSUM bank fill (18*7=126 rows)
MSUB = 3                           # mask shifts per staging tile

F32 = mybir.dt.float32
BF16 = mybir.dt.bfloat16


def _mask_steps(nc, tc, ctx, a1, a2py, cmask):
    """Generator: emits mask-path setup, then yields after each dx-pair
    (135 yields).  Driven interleaved from the cost loop.  The per-shift
    multiplies run on the Pool engine with two dx shifts packed into the
    partition dim ((dd, b, y) -> 96 partitions), halving Pool's per-element
    work versus the 48-partition layout."""
    singles = ctx.enter_context(tc.tile_pool(name="msk_singles", bufs=1))
    m2y_pool = ctx.enter_context(tc.tile_pool(name="msk_m2y", bufs=3))
    mstg_pool = ctx.enter_context(tc.tile_pool(name="msk_stg", bufs=2))

    # mask1 = clip(sum_z alpha1, 0, 1), broadcast over z, duplicated into
    # partitions 48-95 (the dd=1 dx half).  partitions dd*48 + b*12 + y.
    a1t = singles.tile([B * HS, W, D], F32)
    for b in range(B):
        nc.sync.dma_start(a1t[b * HS:(b + 1) * HS, :, :], a1[b])
    mask1 = singles.tile([B * HS, W], F32)
    nc.vector.tensor_reduce(mask1[:, :], a1t[:, :, :],
                            axis=mybir.AxisListType.X, op=mybir.AluOpType.add)
    nc.gpsimd.tensor_scalar_min(mask1[:, :], mask1[:, :], 1.0)
    mask1b = singles.tile([2 * B * HS, W, D], BF16)
    for zi in range(D):
        nc.gpsimd.tensor_copy(mask1b[:B * HS, :, zi], mask1[:, :])
    # engine ops need 32-aligned partition bases, so replicate into the
    # dd=1 half (partitions 48-95) with a DMA.
    nc.sync.dma_start(mask1b[B * HS:, :, :], mask1b[:B * HS, :, :])

    # m2 halos: one-padded mask2, partitions b*20+y' (y' in padded grid).
    # a2py is alpha2 with y pre-padded by 0.125 (padded rows z-sum to 1.0);
    # x/z pads come from the memset(1.0).
    a2t = singles.tile([B * HP, W, D], F32)
    for b in range(B):
        nc.sync.dma_start(a2t[b * HP:(b + 1) * HP, :, :], a2py[b])
    m2sum = singles.tile([B * HP, W], F32)
    nc.vector.tensor_reduce(m2sum[:, :], a2t[:, :, :],
                            axis=mybir.AxisListType.X, op=mybir.AluOpType.add)
    nc.gpsimd.tensor_scalar_min(m2sum[:, :], m2sum[:, :], 1.0)
    # bf16 one-padded halos built directly: plain + z-shifted-by-1 (keeps
    # pair-packed 2x perf mode for odd dz reads).
    m2tb = singles.tile([B * HP, F2X, F2Z], BF16)
    m2tbz = singles.tile([B * HP, F2X, F2Z], BF16)
    nc.gpsimd.memset(m2tb[:, :, :], 1.0)
    nc.gpsimd.memset(m2tbz[:, :, :], 1.0)
    for zi in range(D):
        nc.gpsimd.tensor_copy(m2tb[:, S:S + W, SD + zi], m2sum[:, :])
        nc.gpsimd.tensor_copy(m2tbz[:, S:S + W, SD + zi - 1], m2sum[:, :])

    tiles = {}

    def load(dyi):
        # [96, F2X, F2Z] per z-variant: partitions 0-47 at x-offset 0,
        # partitions 48-95 pre-shifted by one x (so one mul covers dx0 and
        # dx0+1).  The upper half's last x column is stale but never read
        # (pair dx0 <= 6 reads x <= 102 there).
        m2yt = m2y_pool.tile([2 * B * HS, F2X, F2Z], BF16, tag="m2ya")
        m2ytz = m2y_pool.tile([2 * B * HS, F2X, F2Z], BF16, tag="m2yb")
        for src, dst in ((m2tb, m2yt), (m2tbz, m2ytz)):
            for half, xo in ((0, 0), (B * HS, 1)):
                for b in range(B):
                    nc.sync.dma_start(
                        dst[half + b * HS:half + (b + 1) * HS,
                            0:F2X - xo, :],
                        src[b * HP + dyi:b * HP + dyi + HS, xo:F2X, :])
        tiles[dyi] = (m2yt, m2ytz)

    load(0)
    for dyi in range(NSX):
        if dyi + 1 < NSX:
            load(dyi + 1)
        m2yt, m2ytz = tiles.pop(dyi)
        for dzi in range(NSD):
            msrc, dz0 = (m2yt, dzi) if dzi != 1 else (m2ytz, 0)
            for dx0 in range(0, NSX, 2):
                npart = 2 * B * HS if dx0 + 1 < NSX else B * HS
                mstg = mstg_pool.tile([2 * B * HS, W, D], BF16, tag="mstgb")
                nc.gpsimd.tensor_mul(
                    mstg[:npart, :, :], mask1b[:npart, :, :],
                    msrc[:npart, dx0:dx0 + W, dz0:dz0 + D])
                mstf = mstg_pool.tile([2 * B * HS, W, D], F32, tag="mstgf")
                nc.scalar.copy(mstf[:npart, :, :], mstg[:npart, :, :])
                s0 = dzi * NSX2 + dyi * NSX + dx0
                for dd in range(npart // (B * HS)):
                    hbm = cmask[:, s0 + dd].rearrange("b y x z -> b y (x z)")
                    nc.sync.dma_start(
                        hbm, mstf[dd * B * HS:(dd + 1) * B * HS].rearrange(
                            "p x z -> p (x z)"))
                yield


def _build_cost_path(nc, tc, ctx, f1, f2a, f2b, wones, cost, mask_iter):
    """Cost volume: bf16 muls on DVE + Pool, channel-reduce on PE.  Drives
    one mask subgroup per cost group so mask work interleaves on Pool."""
    singles = ctx.enter_context(tc.tile_pool(name="cst_singles", bufs=1))
    f2_pool = ctx.enter_context(tc.tile_pool(name="cst_f2", bufs=2))
    prod_pool = ctx.enter_context(tc.tile_pool(name="cst_prod", bufs=5))
    pprod_pool = ctx.enter_context(tc.tile_pool(name="cst_pprod", bufs=3))
    psum_pool = ctx.enter_context(
        tc.tile_pool(name="cst_psum", bufs=6, space="PSUM"))
    stage_pool = ctx.enter_context(tc.tile_pool(name="cst_stage", bufs=4))

    wt = singles.tile([128, CHUNK], BF16)
    nc.sync.dma_start(wt[:, :], wones[:, :])

    # f1 resident for all b: partition g*64+c (g = y half), free (b, yl, x, z)
    f1t = singles.tile([128, B, GH, W, D], BF16)
    for b in range(B):
        for g in range(2):
            nc.sync.dma_start(
                f1t[64 * g:64 * (g + 1), b, :, :, :],
                f1[b, :, GH * g:GH * (g + 1), :, :].rearrange(
                    "c y x z -> c y (x z)"))

    # Two phases per batch: dz in {0, 2} shifts read the plain f2 copy, the
    # dz=1 phase reads the z-shifted copy.  One f2 halo tile live per phase.
    srcs = (f2a, f2b)
    plan = []
    for b in range(B):
        for src_idx, s_ranges in ((0, [(0, NSX2), (2 * NSX2, 3 * NSX2)]),
                                  (1, [(NSX2, 2 * NSX2)])):
            ph = (b, src_idx)
            for r0, r1 in s_ranges:
                for s0 in range(r0, r1, SGRP):
                    plan.append((ph, b, src_idx, s0, min(SGRP, r1 - s0)))

    # Pool-engine shift assignment: one per group, a second (placed late in
    # the group so its PE deadline is loose) in every other group, to
    # balance DVE vs Pool busy time.  The first two groups of each phase
    # stay off Pool: at a phase boundary Pool has a burst of lookahead
    # muls queued and would deliver the group-start tile late, stalling PE.
    phase_start = {}
    for gi, (ph, b, src_idx, s0, sg) in enumerate(plan):
        phase_start.setdefault(ph, gi)
    pool_sel = {}
    for gi, (ph, b, src_idx, s0, sg) in enumerate(plan):
        sel = set()
        if sg >= 3:
            sel.add(1)
        if gi % 4 == 1 and sg >= 7:
            sel.add(5)
        pool_sel[gi] = sel

    f2_tiles = {}

    def ensure_f2(ph):
        if ph not in f2_tiles:
            b, src_idx = ph
            f2t = f2_pool.tile([128, F2Y, F2X, F2Z], BF16, tag="f2")
            # Issued from the SP queue: the load's slot-WAR wait (cleared
            # only when the outgoing phase finishes reading) must not sit at
            # the ACT queue head, where it would gate the PSUM->SBUF bank
            # copies and stall PE/DVE at every phase boundary.  On SP only
            # output stores queue behind it, and the stage pool absorbs
            # those for several groups.
            for g in range(2):
                nc.sync.dma_start(
                    f2t[64 * g:64 * (g + 1), :, :, :],
                    srcs[src_idx][b, :, GH * g:GH * g + F2Y, :, :])
            f2_tiles[ph] = f2t
        return f2_tiles[ph]

    def shift_view(f2t, s):
        dzi, rem = divmod(s, NSX2)
        dyi, dxi = divmod(rem, NSX)
        dz0 = dzi if dzi != 1 else 0
        return f2t[:, dyi:dyi + GH, dxi:dxi + W, dz0:dz0 + D]

    # dz=+1 shifts (dzi==2) produce an exact zero at output z=D-1 (the f2
    # zero-pad), so their muls only compute z 0..D-2 (all operands stay at
    # even z offsets, keeping bf16 pair packing; dz=-1 would need odd
    # offsets and cannot be trimmed).  On entry to each dzi==2 range the
    # first `bufs` tiles get a z=D-1 memset on the SAME engine queue as
    # their mul; within the range the column is never rewritten, so every
    # later PE read is ordered after its prime via the mul's RAW semaphore
    # plus the producing engine's FIFO.  (CoreSim's race detector tracks
    # byte ownership per tile instance and cannot see this transitive
    # FIFO ordering, so it flags the reuse; the schedule is sound.)
    def emit_mul(eng, streak, nbufs, pt, f1s, f2t, s):
        dzi = s // NSX2
        if dzi == 2:
            if streak[0] < nbufs:
                eng.memset(pt[:, :, :, D - 1:D], 0.0)
                streak[0] += 1
            eng.tensor_mul(pt[:, :, :, 0:D - 1], f1s[:, :, :, 0:D - 1],
                           shift_view(f2t, s)[:, :, :, 0:D - 1])
        else:
            streak[0] = 0
            eng.tensor_mul(pt[:, :, :, :], f1s, shift_view(f2t, s))

    pool_tiles = {}
    dve_streak = [0]
    pool_streak = [0]

    def emit_pool_muls(gi):
        ph, b, src_idx, s0, sg = plan[gi]
        f2t = ensure_f2(ph)
        for si in sorted(pool_sel[gi]):
            pt = pprod_pool.tile([128, GH, W, D], BF16, tag="pprod")
            emit_mul(nc.gpsimd, pool_streak, 3, pt,
                     f1t[:, b, :, :, :], f2t, s0 + si)
            pool_tiles[(gi, si)] = pt

    # First f2 phase ahead of the mask-setup loads on the SP queue: the
    # first cost mul waits on it.
    ensure_f2(plan[0][0])
    # Mask setup next: its DMA loads and small DVE reduces run before the
    # cost stream needs those engines.
    next(mask_iter, None)
    # Pool muls run two groups ahead of their PE consumption so the slower
    # engine never gates TensorE.
    emit_pool_muls(0)
    emit_pool_muls(1)
    # f2 halo loads dispatch from the ACT queue, which runs at the
    # pipeline's pace (~2 groups behind emission); emit each phase's load 6
    # groups before its first use so the ~10us transfer is fully hidden.
    PREFETCH = 13
    for gi, (ph, b, src_idx, s0, sg) in enumerate(plan):
        f2t = ensure_f2(ph)
        if gi + PREFETCH < len(plan):
            ensure_f2(plan[gi + PREFETCH][0])
        if gi + 2 < len(plan):
            emit_pool_muls(gi + 2)
        last_q = NCHUNK * sg - 1
        bank = psum_pool.tile([128, CHUNK], F32)
        for si in range(sg):
            if si in pool_sel[gi]:
                ptile = pool_tiles.pop((gi, si))
            else:
                ptile = prod_pool.tile([128, GH, W, D], BF16, tag="prod")
                emit_mul(nc.vector, dve_streak, 5, ptile,
                         f1t[:, b, :, :, :], f2t, s0 + si)
            ptf = ptile.rearrange("p y x z -> p (y x z)")
            for cch in range(NCHUNK):
                q = si * NCHUNK + cch
                jj, t = divmod(q, 16)
                nc.tensor.matmul(
                    bank[32 * jj:32 * jj + 32, :],
                    wt[:, 32 * t:32 * t + 32],
                    ptf[:, CHUNK * cch:CHUNK * (cch + 1)],
                    start=(t == 0), stop=(t == 15 or q == last_q),
                    tile_position=(0, 32 * jj),
                )
        rows = 18 * sg
        ctile = stage_pool.tile([128, CHUNK], F32)
        nc.scalar.copy(ctile[:rows, :], bank[:rows, :])
        for si in range(sg):
            hbm = cost[b, s0 + si].rearrange(
                "y x z -> (y x z)").rearrange(
                "(g cj ci) -> cj g ci", g=2, cj=NCHUNK)
            nc.sync.dma_start(hbm, ctile[18 * si:18 * si + 18, :])
        next(mask_iter, None)
    # Drain any leftover mask work.
    for _ in mask_iter:
        pass


def build_program():
    nc = bacc.Bacc("TRN2", target_bir_lowering=False, debug=False,
                   num_devices=NCORES)
    f1 = nc.dram_tensor("f1", [B, C, HS, W, D], BF16, kind="ExternalInput").ap()
    f2a = nc.dram_tensor("f2a", [B, C, HP, F2X, F2Z], BF16,
                         kind="ExternalInput").ap()
    f2b = nc.dram_tensor("f2b", [B, C, HP, F2X, F2Z], BF16,
                         kind="ExternalInput").ap()
    a1 = nc.dram_tensor("a1", [B, HS, W, D], F32, kind="ExternalInput").ap()
    a2py = nc.dram_tensor("a2py", [B, HP, W, D], F32,
                          kind="ExternalInput").ap()
    wones = nc.dram_tensor("wones", [128, CHUNK], BF16,
                           kind="ExternalInput").ap()
    cost = nc.dram_tensor("cost", [B, NS, HS, W, D], F32,
                          kind="ExternalOutput").ap()
    cmask = nc.dram_tensor("cmask", [B, NS, HS, W, D], F32,
                           kind="ExternalOutput").ap()

    with tile.TileContext(nc) as tc:
        with ExitStack() as ctx:
            mask_iter = _mask_steps(nc, tc, ctx, a1, a2py, cmask)
            _build_cost_path(nc, tc, ctx, f1, f2a, f2b, wones, cost,
                             mask_iter)
    nc.compile()
    return nc


def make_wones() -> np.ndarray:
    """Ones-pattern PE weights: column t*32 + r is 1 on channel-group g rows
    iff r == 2t + g, so accumulation step t lands chunk t's two group sums on
    strip-local rows 2t, 2t+1."""
    w = np.zeros((128, CHUNK), np.float32)
    for t in range(16):
        for g in range(2):
            w[g * 64:(g + 1) * 64, t * 32 + 2 * t + g] = 1.0
    return w.astype(ml_dtypes.bfloat16)


def prepare_host_inputs(mpi1_features, mpi1_alpha, mpi2_features, mpi2_alpha):
    f1 = np.asarray(mpi1_features, np.float32)
    a1 = np.asarray(mpi1_alpha, np.float32)[:, 0]          # [B, H, W, D]
    f2 = np.asarray(mpi2_features, np.float32)
    a2 = np.asarray(mpi2_alpha, np.float32)[:, 0]

    bf = ml_dtypes.bfloat16
    # Fold the 1/64 channel mean into f1's bf16 cast (exact, power of two).
    f1b = (f1 * (1.0 / C)).astype(bf)
    f2p = np.zeros((B, C, H + 2 * S, W + 2 * S, D + 2 * SD), np.float32)
    f2p[:, :, S:S + H, S:S + W, SD:SD + D] = f2
    f2pa = f2p.astype(bf)
    f2pb = np.zeros_like(f2p)
    f2pb[..., :-1] = f2p[..., 1:]                           # z-shift by +1
    f2pb = f2pb.astype(bf)
    # alpha2 with y padded by 0.125: padded rows z-sum to exactly 1.0.
    a2py = np.full((B, H + 2 * S, W, D), 0.125, np.float32)
    a2py[:, S:S + H] = a2
    return f1b, f2pa, f2pb, a1, a2py


def make_in_maps(f1b, f2pa, f2pb, a1, a2py):
    wones = make_wones()
    in_maps = []
    for k in range(NCORES):
        y0 = k * HS
        in_maps.append({
            "f1": np.ascontiguousarray(f1b[:, :, y0:y0 + HS]),
            "f2a": np.ascontiguousarray(f2pa[:, :, y0:y0 + HP]),
            "f2b": np.ascontiguousarray(f2pb[:, :, y0:y0 + HP]),
            "a1": np.ascontiguousarray(a1[:, y0:y0 + HS]),
            "a2py": np.ascontiguousarray(a2py[:, y0:y0 + HP]),
            "wones": wones,
        })
    return in_maps


_PROGRAM_CACHE = {}


def kernel(mpi1_features, mpi1_alpha, mpi2_features, mpi2_alpha,
           _trace=False, _trace_kwargs=None):
    if "nc" not in _PROGRAM_CACHE:
        _PROGRAM_CACHE["nc"] = build_program()
    nc = _PROGRAM_CACHE["nc"]
    in_maps = make_in_maps(*prepare_host_inputs(
        mpi1_features, mpi1_alpha, mpi2_features, mpi2_alpha))
    res = run_bass_kernel_spmd(nc, in_maps, list(range(NCORES)),
                               trace=_trace, **(_trace_kwargs or {}))
    cost = np.concatenate([res.results[k]["cost"] for k in range(NCORES)],
                          axis=2)
    cmask = np.concatenate([res.results[k]["cmask"] for k in range(NCORES)],
                           axis=2)
    if _trace:
        kernel.last_results = res
    return cost, cmask



# revision 3
# speedup vs baseline: 1.0368x; 1.0368x over previous
"""Trainium2 Bass kernel for a FlowNet-style MPI correlation layer — Gram
matmul formulation.

Reference (per batch b, shift s=(k,i,j), position p=(y,x,z)):
    cost[b,s,p]  = mean_c f1[b,c,p] * f2pad[b,c,p+delta_s]        (243 shifts)
    cmask[b,s,p] = clip(mask1[b,p] * m2pad[b,p+delta_s], 0, 1)

Key idea: for a fixed f2 row (b, ys, zs), the x-shifted channel dot products
against every f1 row that references it are columns of a small Gram matrix
    G[x', n] = sum_c f2[b,c,ys,x',zs] * f1[b,c,y_n,x_n,z_n]
whose 9-wide band x' - x_n = dxi in [0..8] is exactly the cost output.  The
multiplies therefore run on the TensorEngine (stationary = f2 x'-windows,
moving = f1 columns) instead of the DVE, which was the 1.9 ms bottleneck of
the elementwise formulation.

Per core (h-sharded, 12 y-rows + 8-halo):
  - Groups (b, ys in 0..19, zs in 0..7).  Four column-tiles j (tile_position
    (0,32j)) hold f2[c64, x' in 24j..24j+32] stationary; each streams the f1
    block x in 24j..24j+24 for all valid (z = zs-dzi, y = ys-4-dyi') combos,
    N = ncomb*24 columns shared across the 4 concurrent tiles.  PSUM gets
    [128 = 4 strips x 32 x'-rows, N].
  - The band cannot be extracted on-chip (no engine or DMA primitive crosses
    partition<->free at 4-byte granularity), so the dense PSUM block is
    evacuated to SBUF as bf16 by the DVE (~228K free-cycles total), DMA'd to
    an HBM scratch `gsup`, and the 9-diagonal band is gathered on the host
    with one strided-view copy + one precomputed fancy scatter per core.
  - Mask path: unchanged from the elementwise kernel (Pool-engine bf16 muls,
    two dx shifts packed per op), except cmask is stored as bf16 and
    upcast on the host.
"""

import numpy as np
import ml_dtypes
from contextlib import ExitStack

import concourse.bacc as bacc
import concourse.tile as tile
from concourse import mybir
from concourse.bass_utils import run_bass_kernel_spmd

# Problem shape (hardcoded per contest contract).
B, C, H, W, D = 4, 64, 96, 96, 8
S, SD = 4, 1                       # spatial / depth search range
NSX = 2 * S + 1                    # 9 shifts per spatial axis
NSD = 2 * SD + 1                   # 3 depth shifts
NS = NSX * NSX * NSD               # 243 total shifts
NSX2 = NSX * NSX
NCORES = 8
HS = H // NCORES                   # 12 rows of y per core
HP = HS + 2 * S                    # 20 = y rows incl halo
XP = W + 2 * S                     # 104 = x cols incl pad
T = 24                             # f1 x-block per column tile
MT = 32                            # PSUM rows per tile (= T + 2*S)
NT = 4                             # column tiles
MAXCOMB = 21                       # combos per PSUM fill (21*24 = 504 <= 512)
NST = NSX * 5 * NSD                # 135 mask product steps (dyi, dx-pair, dzi)

F32 = mybir.dt.float32
BF16 = mybir.dt.bfloat16

MSUB = 3


def make_schedule():
    """Fills for the cost path.  Each fill: dict(b, ys, zs, z0, nz, y0, ny,
    cc0) where (z, y) combos are ordered z-major then y ascending and cc0 is
    the global combo cursor.  Mirrors exactly what the device program emits
    and what the host gather assumes."""
    fills = []
    cc = 0
    for b in range(B):
        for ys in range(HP):
            y0 = max(0, ys - 2 * S)
            y1 = min(HS, ys + 1)
            ny = y1 - y0
            if ny <= 0:
                continue
            for zs in range(D):
                z0 = max(0, zs - SD)
                z1 = min(D, zs + SD + 1)
                nz = z1 - z0
                # split z-runs so each fill has <= MAXCOMB combos
                zr = []
                if nz * ny <= MAXCOMB:
                    zr.append((z0, nz))
                else:
                    step = max(1, MAXCOMB // ny)
                    zz = z0
                    while zz < z1:
                        n = min(step, z1 - zz)
                        zr.append((zz, n))
                        zz += n
                for (fz0, fnz) in zr:
                    ncomb = fnz * ny
                    fills.append(dict(b=b, ys=ys, zs=zs, z0=fz0, nz=fnz,
                                      y0=y0, ny=ny, cc0=cc))
                    cc += ncomb
    return fills, cc


_SCHED_CACHE = {}


def schedule():
    if "s" not in _SCHED_CACHE:
        _SCHED_CACHE["s"] = make_schedule()
    return _SCHED_CACHE["s"]


def _mask_steps(nc, tc, ctx, a1, a2py, cm_sc):
    """Mask path (generator, one yield per dx-pair; 45 yields).  Like the
    elementwise kernel's mask path, but each Pool-staged product block is
    DMA'd contiguously (bf16) to the cm_sc HBM scratch via the Pool SWDGE
    (one 2-dim DMA per step); the host scatters it into cmask layout."""
    singles = ctx.enter_context(tc.tile_pool(name="msk_singles", bufs=1))
    m2y_pool = ctx.enter_context(tc.tile_pool(name="msk_m2y", bufs=3))
    mstg_pool = ctx.enter_context(tc.tile_pool(name="msk_stg", bufs=4))

    F2X, F2Z = XP, D + 2 * SD

    a1t = singles.tile([B * HS, W, D], F32)
    for b in range(B):
        nc.sync.dma_start(a1t[b * HS:(b + 1) * HS, :, :], a1[b])
    mask1 = singles.tile([B * HS, W], F32)
    nc.vector.tensor_reduce(mask1[:, :], a1t[:, :, :],
                            axis=mybir.AxisListType.X, op=mybir.AluOpType.add)
    nc.gpsimd.tensor_scalar_min(mask1[:, :], mask1[:, :], 1.0)
    mask1b = singles.tile([2 * B * HS, W, D], BF16)
    for zi in range(D):
        nc.gpsimd.tensor_copy(mask1b[:B * HS, :, zi], mask1[:, :])
    nc.sync.dma_start(mask1b[B * HS:, :, :], mask1b[:B * HS, :, :])

    a2t = singles.tile([B * HP, W, D], F32)
    for b in range(B):
        nc.sync.dma_start(a2t[b * HP:(b + 1) * HP, :, :], a2py[b])
    m2sum = singles.tile([B * HP, W], F32)
    nc.vector.tensor_reduce(m2sum[:, :], a2t[:, :, :],
                            axis=mybir.AxisListType.X, op=mybir.AluOpType.add)
    nc.gpsimd.tensor_scalar_min(m2sum[:, :], m2sum[:, :], 1.0)
    m2tb = singles.tile([B * HP, F2X, F2Z], BF16)
    m2tbz = singles.tile([B * HP, F2X, F2Z], BF16)
    nc.gpsimd.memset(m2tb[:, :, :], 1.0)
    nc.gpsimd.memset(m2tbz[:, :, :], 1.0)
    for zi in range(D):
        nc.gpsimd.tensor_copy(m2tb[:, S:S + W, SD + zi], m2sum[:, :])
        nc.gpsimd.tensor_copy(m2tbz[:, S:S + W, SD + zi - 1], m2sum[:, :])

    tiles = {}

    def load(dyi):
        m2yt = m2y_pool.tile([2 * B * HS, F2X, F2Z], BF16, tag="m2ya")
        m2ytz = m2y_pool.tile([2 * B * HS, F2X, F2Z], BF16, tag="m2yb")
        for src, dst in ((m2tb, m2yt), (m2tbz, m2ytz)):
            for half, xo in ((0, 0), (B * HS, 1)):
                for b in range(B):
                    nc.sync.dma_start(
                        dst[half + b * HS:half + (b + 1) * HS,
                            0:F2X - xo, :],
                        src[b * HP + dyi:b * HP + dyi + HS, xo:F2X, :])
        tiles[dyi] = (m2yt, m2ytz)

    load(0)
    st = 0
    for dyi in range(NSX):
        if dyi + 1 < NSX:
            load(dyi + 1)
        m2yt, m2ytz = tiles.pop(dyi)
        for dx0 in range(0, NSX, 2):
            npart = 2 * B * HS if dx0 + 1 < NSX else B * HS
            for dzi in range(NSD):
                msrc, dz0 = (m2yt, dzi) if dzi != 1 else (m2ytz, 0)
                mstg = mstg_pool.tile([2 * B * HS, W, D], BF16, tag="mstgb")
                meng = nc.gpsimd if dzi == 1 else nc.vector
                meng.tensor_mul(
                    mstg[:npart, :, :], mask1b[:npart, :, :],
                    msrc[:npart, dx0:dx0 + W, dz0:dz0 + D])
                nc.gpsimd.dma_start(
                    cm_sc[st, 0:npart].rearrange("p x z -> p (x z)"),
                    mstg[:npart].rearrange("p x z -> p (x z)"))
                st += 1
            yield


def _build_cost_path(nc, tc, ctx, f1, f2p, gsup, mask_iter):
    """Cost volume via banded-Gram matmuls on the TensorEngine."""
    fills, nctot = schedule()
    f1_pool = ctx.enter_context(tc.tile_pool(name="cst_f1", bufs=2))
    f2_pool = ctx.enter_context(tc.tile_pool(name="cst_f2", bufs=2))
    psum_pool = ctx.enter_context(
        tc.tile_pool(name="cst_psum", bufs=8, space="PSUM"))
    stage_pool = ctx.enter_context(tc.tile_pool(name="cst_stage", bufs=10))
    SCAP = 2048                    # stage cols per output DMA

    btiles = {}

    def ensure_b(b):
        if b not in btiles:
            f1t = f1_pool.tile([C, HS, W, D], BF16, tag="f1")
            nc.sync.dma_start(f1t[:, :, :, :], f1[b])
            f2t = f2_pool.tile([C, HP, XP, D], BF16, tag="f2")
            nc.sync.dma_start(f2t[:, :, :, :], f2p[b])
            btiles[b] = (f1t, f2t)
        return btiles[b]

    ensure_b(0)
    next(mask_iter, None)
    PREFETCH_AT = {}
    # prefetch next b's tiles a bit before the phase boundary
    for fi, f in enumerate(fills):
        if f["b"] + 1 < B:
            PREFETCH_AT[f["b"] + 1] = fi  # last fill index before switch...
    # simpler: find first fill index of each b, prefetch 12 fills earlier
    first_of_b = {}
    for fi, f in enumerate(fills):
        first_of_b.setdefault(f["b"], fi)

    mask_period = max(1, len(fills) // 50)

    stg = None
    scur = 0
    gbase = 0

    def flush():
        nonlocal stg, scur
        if stg is not None and scur > 0:
            nc.scalar.dma_start(gsup[:, gbase:gbase + scur], stg[:, 0:scur])
        stg = None
        scur = 0

    for fi, f in enumerate(fills):
        b, ys, zs = f["b"], f["ys"], f["zs"]
        z0, nz, y0, ny = f["z0"], f["nz"], f["y0"], f["ny"]
        f1t, f2t = ensure_b(b)
        for nb, fstart in first_of_b.items():
            if nb > b and fi == max(0, fstart - 60):
                ensure_b(nb)
        L = nz * ny * T
        bank = psum_pool.tile([128, 512], F32)
        for j in range(NT):
            rhs = f1t[:, y0:y0 + ny, T * j:T * j + T, z0:z0 + nz].rearrange(
                "c y x z -> c z y x")
            nc.tensor.matmul(
                bank[MT * j:MT * j + MT, 0:L],
                f2t[:, ys, T * j:T * j + MT, zs],
                rhs,
                start=True, stop=True,
                tile_position=(0, MT * j),
            )
        if stg is not None and scur + L > SCAP:
            flush()
        if stg is None:
            stg = stage_pool.tile([128, SCAP], BF16)
            gbase = f["cc0"] * T
        nc.vector.tensor_copy(stg[:, scur:scur + L], bank[:, 0:L])
        scur += L
        if fi % mask_period == 0:
            next(mask_iter, None)
    flush()
    for _ in mask_iter:
        pass


def build_program():
    fills, nctot = schedule()
    nc = bacc.Bacc("TRN2", target_bir_lowering=False, debug=False,
                   num_devices=NCORES)
    f1 = nc.dram_tensor("f1", [B, C, HS, W, D], BF16, kind="ExternalInput").ap()
    f2p = nc.dram_tensor("f2p", [B, C, HP, XP, D], BF16,
                         kind="ExternalInput").ap()
    a1 = nc.dram_tensor("a1", [B, HS, W, D], F32, kind="ExternalInput").ap()
    a2py = nc.dram_tensor("a2py", [B, HP, W, D], F32,
                          kind="ExternalInput").ap()
    gsup = nc.dram_tensor("gsup", [128, nctot * T], BF16,
                          kind="ExternalOutput").ap()
    cm_sc = nc.dram_tensor("cm_sc", [NST, 2 * B * HS, W, D], BF16,
                           kind="ExternalOutput").ap()

    with tile.TileContext(nc) as tc:
        with ExitStack() as ctx:
            mask_iter = _mask_steps(nc, tc, ctx, a1, a2py, cm_sc)
            _build_cost_path(nc, tc, ctx, f1, f2p, gsup, mask_iter)
    nc.compile()
    return nc


def prepare_host_inputs(mpi1_features, mpi1_alpha, mpi2_features, mpi2_alpha):
    f1 = np.asarray(mpi1_features, np.float32)
    a1 = np.asarray(mpi1_alpha, np.float32)[:, 0]          # [B, H, W, D]
    f2 = np.asarray(mpi2_features, np.float32)
    a2 = np.asarray(mpi2_alpha, np.float32)[:, 0]

    bf = ml_dtypes.bfloat16
    # Fold the 1/64 channel mean into f1's bf16 cast (exact: power of two).
    f1b = (f1 * (1.0 / C)).astype(bf)
    f2p = np.zeros((B, C, H + 2 * S, W + 2 * S, D), np.float32)
    f2p[:, :, S:S + H, S:S + W, :] = f2
    f2pb = f2p.astype(bf)
    # alpha2 with y padded by 0.125: padded rows z-sum to exactly 1.0.
    a2py = np.full((B, H + 2 * S, W, D), 0.125, np.float32)
    a2py[:, S:S + H] = a2
    return f1b, f2pb, a1, a2py


def make_in_maps(f1b, f2pb, a1, a2py):
    in_maps = []
    for k in range(NCORES):
        y0 = k * HS
        in_maps.append({
            "f1": np.ascontiguousarray(f1b[:, :, y0:y0 + HS]),
            "f2p": np.ascontiguousarray(f2pb[:, :, y0:y0 + HP]),
            "a1": np.ascontiguousarray(a1[:, y0:y0 + HS]),
            "a2py": np.ascontiguousarray(a2py[:, y0:y0 + HP]),
        })
    return in_maps


def _gather_indices():
    """Flat int32 scatter indices: band[jt, dxi, cc, xf] -> cost[b,s,y,x,z]
    (per-core local cost of shape [B, NS, HS, W, D])."""
    if "idx" in _SCHED_CACHE:
        return _SCHED_CACHE["idx"]
    fills, nctot = schedule()
    b_a = np.empty(nctot, np.int64)
    y_a = np.empty(nctot, np.int64)
    z_a = np.empty(nctot, np.int64)
    ys_a = np.empty(nctot, np.int64)
    zs_a = np.empty(nctot, np.int64)
    for f in fills:
        cc0, ny, nz = f["cc0"], f["ny"], f["nz"]
        n = ny * nz
        zz = np.repeat(np.arange(f["z0"], f["z0"] + nz), ny)
        yy = np.tile(np.arange(f["y0"], f["y0"] + ny), nz)
        b_a[cc0:cc0 + n] = f["b"]
        y_a[cc0:cc0 + n] = yy
        z_a[cc0:cc0 + n] = zz
        ys_a[cc0:cc0 + n] = f["ys"]
        zs_a[cc0:cc0 + n] = f["zs"]
    jt = np.arange(NT)[:, None, None, None]
    dxi = np.arange(NSX)[None, :, None, None]
    cc = slice(None)
    xf = np.arange(T)[None, None, None, :]
    bb = b_a[None, None, :, None]
    yy = y_a[None, None, :, None]
    zz = z_a[None, None, :, None]
    s = ((zs_a - z_a + 1)[None, None, :, None] * NSX2
         + (ys_a - y_a)[None, None, :, None] * NSX + dxi)
    x = T * jt + xf
    flat = (((bb * NS + s) * HS + yy) * W + x) * D + zz
    idx = np.ascontiguousarray(flat.astype(np.int32).ravel())
    _SCHED_CACHE["idx"] = idx
    return idx


def _gather_core(gsup_core):
    """gsup [128, nctot*T] bf16 -> band [NT, 9, nctot, T] f32 via one strided
    view copy."""
    fills, nctot = schedule()
    a = np.ascontiguousarray(gsup_core)
    totc = nctot * T
    es = a.dtype.itemsize
    band = np.lib.stride_tricks.as_strided(
        a, shape=(NT, NSX, nctot, T),
        strides=(MT * totc * es, totc * es, T * es, totc * es + es))
    return band.astype(np.float32)


_PROGRAM_CACHE = {}


def kernel(mpi1_features, mpi1_alpha, mpi2_features, mpi2_alpha,
           _trace=False, _trace_kwargs=None):
    if "nc" not in _PROGRAM_CACHE:
        _PROGRAM_CACHE["nc"] = build_program()
    nc = _PROGRAM_CACHE["nc"]
    in_maps = make_in_maps(*prepare_host_inputs(
        mpi1_features, mpi1_alpha, mpi2_features, mpi2_alpha))
    res = run_bass_kernel_spmd(nc, in_maps, list(range(NCORES)),
                               trace=_trace, **(_trace_kwargs or {}))
    idx = _gather_indices()
    costs = []
    cmasks = []
    for k in range(NCORES):
        band = _gather_core(res.results[k]["gsup"])
        c = np.zeros(B * NS * HS * W * D, np.float32)
        c[idx] = band.ravel()
        costs.append(c.reshape(B, NS, HS, W, D))
        sc = np.asarray(res.results[k]["cm_sc"], np.float32)
        cm = np.empty((B, NS, HS, W, D), np.float32)
        for st in range(NST):
            dyi, rem = divmod(st, 5 * NSD)
            dxp, dzi = divmod(rem, NSD)
            dx0 = 2 * dxp
            s0 = dzi * NSX2 + dyi * NSX + dx0
            nds = 2 if dx0 + 1 < NSX else 1
            for dd in range(nds):
                cm[:, s0 + dd] = sc[st, dd * B * HS:(dd + 1) * B * HS
                                    ].reshape(B, HS, W, D)
        cmasks.append(cm)
    cost = np.concatenate(costs, axis=2)
    cmask = np.concatenate(cmasks, axis=2)
    if _trace:
        kernel.last_results = res
    return cost, cmask


# revision 4
# speedup vs baseline: 1.2567x; 1.2120x over previous
"""Trainium2 Bass kernel for a FlowNet-style MPI correlation layer — Gram
matmul formulation.

Reference (per batch b, shift s=(k,i,j), position p=(y,x,z)):
    cost[b,s,p]  = mean_c f1[b,c,p] * f2pad[b,c,p+delta_s]        (243 shifts)
    cmask[b,s,p] = clip(mask1[b,p] * m2pad[b,p+delta_s], 0, 1)

Key idea: for a fixed f2 row (b, ys, zs), the x-shifted channel dot products
against every f1 row that references it are columns of a small Gram matrix
    G[x', n] = sum_c f2[b,c,ys,x',zs] * f1[b,c,y_n,x_n,z_n]
whose 9-wide band x' - x_n = dxi in [0..8] is exactly the cost output.  The
multiplies therefore run on the TensorEngine (stationary = f2 x'-windows,
moving = f1 columns) instead of the DVE, which was the 1.9 ms bottleneck of
the elementwise formulation.

Per core (h-sharded, 12 y-rows + 8-halo):
  - Groups (b, ys in 0..19, zs in 0..7).  Four column-tiles j (tile_position
    (0,32j)) hold f2[c64, x' in 24j..24j+32] stationary; each streams the f1
    block x in 24j..24j+24 for all valid (z = zs-dzi, y = ys-4-dyi') combos,
    N = ncomb*24 columns shared across the 4 concurrent tiles.  PSUM gets
    [128 = 4 strips x 32 x'-rows, N].
  - The band cannot be extracted on-chip (no engine or DMA primitive crosses
    partition<->free at 4-byte granularity), so the dense PSUM block is
    evacuated to SBUF as bf16 by the DVE (~228K free-cycles total), DMA'd to
    an HBM scratch `gsup`, and the 9-diagonal band is gathered on the host
    with one strided-view copy + one precomputed fancy scatter per core.
  - Mask path: unchanged from the elementwise kernel (Pool-engine bf16 muls,
    two dx shifts packed per op), except cmask is stored as bf16 and
    upcast on the host.
"""

import numpy as np
import ml_dtypes
from contextlib import ExitStack

import concourse.bacc as bacc
import concourse.tile as tile
from concourse import mybir
from concourse.bass_utils import run_bass_kernel_spmd

# Problem shape (hardcoded per contest contract).
B, C, H, W, D = 4, 64, 96, 96, 8
S, SD = 4, 1                       # spatial / depth search range
NSX = 2 * S + 1                    # 9 shifts per spatial axis
NSD = 2 * SD + 1                   # 3 depth shifts
NS = NSX * NSX * NSD               # 243 total shifts
NSX2 = NSX * NSX
NCORES = 8
HS = H // NCORES                   # 12 rows of y per core
HP = HS + 2 * S                    # 20 = y rows incl halo
XP = W + 2 * S                     # 104 = x cols incl pad
T = 24                             # f1 x-block per column tile
MT = 32                            # PSUM rows per tile (= T + 2*S)
NT = 4                             # column tiles
MAXCOMB = 21                       # combos per PSUM fill (21*24 = 504 <= 512)
NST = NSX * 5 * NSD                # 135 mask product steps (dyi, dx-pair, dzi)

F32 = mybir.dt.float32
BF16 = mybir.dt.bfloat16

MSUB = 3


def make_schedule():
    """Fills for the cost path.  Each fill: dict(b, ys, zs, z0, nz, y0, ny,
    cc0) where (z, y) combos are ordered z-major then y ascending and cc0 is
    the global combo cursor.  Mirrors exactly what the device program emits
    and what the host gather assumes."""
    fills = []
    cc = 0
    for b in range(B):
        for ys in range(HP):
            y0 = max(0, ys - 2 * S)
            y1 = min(HS, ys + 1)
            ny = y1 - y0
            if ny <= 0:
                continue
            for zs in range(D):
                z0 = max(0, zs - SD)
                z1 = min(D, zs + SD + 1)
                nz = z1 - z0
                # split z-runs so each fill has <= MAXCOMB combos
                zr = []
                if nz * ny <= MAXCOMB:
                    zr.append((z0, nz))
                else:
                    step = max(1, MAXCOMB // ny)
                    zz = z0
                    while zz < z1:
                        n = min(step, z1 - zz)
                        zr.append((zz, n))
                        zz += n
                for (fz0, fnz) in zr:
                    ncomb = fnz * ny
                    fills.append(dict(b=b, ys=ys, zs=zs, z0=fz0, nz=fnz,
                                      y0=y0, ny=ny, cc0=cc))
                    cc += ncomb
    return fills, cc


_SCHED_CACHE = {}


def schedule():
    if "s" not in _SCHED_CACHE:
        _SCHED_CACHE["s"] = make_schedule()
    return _SCHED_CACHE["s"]


def _mask_steps(nc, tc, ctx, a1, a2py, cm_sc):
    """Mask path (generator, one yield per dx-pair; 45 yields).  Like the
    elementwise kernel's mask path, but each Pool-staged product block is
    DMA'd contiguously (bf16) to the cm_sc HBM scratch via the Pool SWDGE
    (one 2-dim DMA per step); the host scatters it into cmask layout."""
    singles = ctx.enter_context(tc.tile_pool(name="msk_singles", bufs=1))
    m2y_pool = ctx.enter_context(tc.tile_pool(name="msk_m2y", bufs=3))
    mstg_pool = ctx.enter_context(tc.tile_pool(name="msk_stg", bufs=4))

    F2X, F2Z = XP, D + 2 * SD

    a1t = singles.tile([B * HS, W, D], F32)
    for b in range(B):
        nc.sync.dma_start(a1t[b * HS:(b + 1) * HS, :, :], a1[b])
    mask1 = singles.tile([B * HS, W], F32)
    nc.vector.tensor_reduce(mask1[:, :], a1t[:, :, :],
                            axis=mybir.AxisListType.X, op=mybir.AluOpType.add)
    nc.gpsimd.tensor_scalar_min(mask1[:, :], mask1[:, :], 1.0)
    mask1b = singles.tile([2 * B * HS, W, D], BF16)
    for zi in range(D):
        nc.gpsimd.tensor_copy(mask1b[:B * HS, :, zi], mask1[:, :])
    nc.sync.dma_start(mask1b[B * HS:, :, :], mask1b[:B * HS, :, :])

    a2t = singles.tile([B * HP, W, D], F32)
    for b in range(B):
        nc.sync.dma_start(a2t[b * HP:(b + 1) * HP, :, :], a2py[b])
    m2sum = singles.tile([B * HP, W], F32)
    nc.vector.tensor_reduce(m2sum[:, :], a2t[:, :, :],
                            axis=mybir.AxisListType.X, op=mybir.AluOpType.add)
    nc.gpsimd.tensor_scalar_min(m2sum[:, :], m2sum[:, :], 1.0)
    m2tb = singles.tile([B * HP, F2X, F2Z], BF16)
    m2tbz = singles.tile([B * HP, F2X, F2Z], BF16)
    nc.gpsimd.memset(m2tb[:, :, :], 1.0)
    nc.gpsimd.memset(m2tbz[:, :, :], 1.0)
    for zi in range(D):
        nc.gpsimd.tensor_copy(m2tb[:, S:S + W, SD + zi], m2sum[:, :])
        nc.gpsimd.tensor_copy(m2tbz[:, S:S + W, SD + zi - 1], m2sum[:, :])

    tiles = {}

    def load(dyi):
        m2yt = m2y_pool.tile([2 * B * HS, F2X, F2Z], BF16, tag="m2ya")
        m2ytz = m2y_pool.tile([2 * B * HS, F2X, F2Z], BF16, tag="m2yb")
        for src, dst in ((m2tb, m2yt), (m2tbz, m2ytz)):
            for half, xo in ((0, 0), (B * HS, 1)):
                for b in range(B):
                    nc.sync.dma_start(
                        dst[half + b * HS:half + (b + 1) * HS,
                            0:F2X - xo, :],
                        src[b * HP + dyi:b * HP + dyi + HS, xo:F2X, :])
        tiles[dyi] = (m2yt, m2ytz)

    load(0)
    st = 0
    for dyi in range(NSX):
        if dyi + 1 < NSX:
            load(dyi + 1)
        m2yt, m2ytz = tiles.pop(dyi)
        for dx0 in range(0, NSX, 2):
            npart = 2 * B * HS if dx0 + 1 < NSX else B * HS
            for dzi in range(NSD):
                msrc, dz0 = (m2yt, dzi) if dzi != 1 else (m2ytz, 0)
                mstg = mstg_pool.tile([2 * B * HS, W, D], BF16, tag="mstgb")
                meng = nc.gpsimd if dzi == 1 else nc.vector
                meng.tensor_mul(
                    mstg[:npart, :, :], mask1b[:npart, :, :],
                    msrc[:npart, dx0:dx0 + W, dz0:dz0 + D])
                nc.gpsimd.dma_start(
                    cm_sc[st, 0:npart].rearrange("p x z -> p (x z)"),
                    mstg[:npart].rearrange("p x z -> p (x z)"))
                st += 1
            yield


def _build_cost_path(nc, tc, ctx, f1, f2p, gsup, mask_iter):
    """Cost volume via banded-Gram matmuls on the TensorEngine."""
    fills, nctot = schedule()
    f1_pool = ctx.enter_context(tc.tile_pool(name="cst_f1", bufs=2))
    f2_pool = ctx.enter_context(tc.tile_pool(name="cst_f2", bufs=2))
    psum_pool = ctx.enter_context(
        tc.tile_pool(name="cst_psum", bufs=8, space="PSUM"))
    stage_pool = ctx.enter_context(tc.tile_pool(name="cst_stage", bufs=8))
    SCAP = 3072                    # stage cols per output DMA

    btiles = {}

    def ensure_b(b):
        if b not in btiles:
            f1t = f1_pool.tile([C, HS, W, D], BF16, tag="f1")
            nc.sync.dma_start(f1t[:, :, :, :], f1[b])
            f2t = f2_pool.tile([C, HP, XP, D], BF16, tag="f2")
            nc.sync.dma_start(f2t[:, :, :, :], f2p[b])
            btiles[b] = (f1t, f2t)
        return btiles[b]

    ensure_b(0)
    next(mask_iter, None)
    PREFETCH_AT = {}
    # prefetch next b's tiles a bit before the phase boundary
    for fi, f in enumerate(fills):
        if f["b"] + 1 < B:
            PREFETCH_AT[f["b"] + 1] = fi  # last fill index before switch...
    # simpler: find first fill index of each b, prefetch 12 fills earlier
    first_of_b = {}
    for fi, f in enumerate(fills):
        first_of_b.setdefault(f["b"], fi)

    mask_period = max(1, len(fills) // 50)

    stg = None
    scur = 0
    gbase = 0

    def flush():
        nonlocal stg, scur
        if stg is not None and scur > 0:
            nc.scalar.dma_start(gsup[:, gbase:gbase + scur], stg[:, 0:scur])
        stg = None
        scur = 0

    for fi, f in enumerate(fills):
        b, ys, zs = f["b"], f["ys"], f["zs"]
        z0, nz, y0, ny = f["z0"], f["nz"], f["y0"], f["ny"]
        f1t, f2t = ensure_b(b)
        for nb, fstart in first_of_b.items():
            if nb > b and fi == max(0, fstart - 60):
                ensure_b(nb)
        L = nz * ny * T
        bank = psum_pool.tile([128, 512], F32)
        for j in range(NT):
            rhs = f1t[:, y0:y0 + ny, T * j:T * j + T, z0:z0 + nz].rearrange(
                "c y x z -> c z y x")
            nc.tensor.matmul(
                bank[MT * j:MT * j + MT, 0:L],
                f2t[:, ys, T * j:T * j + MT, zs],
                rhs,
                start=True, stop=True,
                tile_position=(0, MT * j),
            )
        if stg is not None and scur + L > SCAP:
            flush()
        if stg is None:
            stg = stage_pool.tile([128, SCAP], BF16)
            gbase = f["cc0"] * T
        nc.vector.tensor_copy(stg[:, scur:scur + L], bank[:, 0:L])
        scur += L
        if fi % mask_period == 0:
            next(mask_iter, None)
    flush()
    for _ in mask_iter:
        pass


def build_program():
    fills, nctot = schedule()
    nc = bacc.Bacc("TRN2", target_bir_lowering=False, debug=False,
                   num_devices=NCORES)
    f1 = nc.dram_tensor("f1", [B, C, HS, W, D], BF16, kind="ExternalInput").ap()
    f2p = nc.dram_tensor("f2p", [B, C, HP, XP, D], BF16,
                         kind="ExternalInput").ap()
    a1 = nc.dram_tensor("a1", [B, HS, W, D], F32, kind="ExternalInput").ap()
    a2py = nc.dram_tensor("a2py", [B, HP, W, D], F32,
                          kind="ExternalInput").ap()
    gsup = nc.dram_tensor("gsup", [128, nctot * T], BF16,
                          kind="ExternalOutput").ap()
    cm_sc = nc.dram_tensor("cm_sc", [NST, 2 * B * HS, W, D], BF16,
                           kind="ExternalOutput").ap()

    with tile.TileContext(nc) as tc:
        with ExitStack() as ctx:
            mask_iter = _mask_steps(nc, tc, ctx, a1, a2py, cm_sc)
            _build_cost_path(nc, tc, ctx, f1, f2p, gsup, mask_iter)
    nc.compile()
    return nc


def prepare_host_inputs(mpi1_features, mpi1_alpha, mpi2_features, mpi2_alpha):
    f1 = np.asarray(mpi1_features, np.float32)
    a1 = np.asarray(mpi1_alpha, np.float32)[:, 0]          # [B, H, W, D]
    f2 = np.asarray(mpi2_features, np.float32)
    a2 = np.asarray(mpi2_alpha, np.float32)[:, 0]

    bf = ml_dtypes.bfloat16
    # Fold the 1/64 channel mean into f1's bf16 cast (exact: power of two).
    f1b = (f1 * (1.0 / C)).astype(bf)
    f2p = np.zeros((B, C, H + 2 * S, W + 2 * S, D), np.float32)
    f2p[:, :, S:S + H, S:S + W, :] = f2
    f2pb = f2p.astype(bf)
    # alpha2 with y padded by 0.125: padded rows z-sum to exactly 1.0.
    a2py = np.full((B, H + 2 * S, W, D), 0.125, np.float32)
    a2py[:, S:S + H] = a2
    return f1b, f2pb, a1, a2py


def make_in_maps(f1b, f2pb, a1, a2py):
    in_maps = []
    for k in range(NCORES):
        y0 = k * HS
        in_maps.append({
            "f1": np.ascontiguousarray(f1b[:, :, y0:y0 + HS]),
            "f2p": np.ascontiguousarray(f2pb[:, :, y0:y0 + HP]),
            "a1": np.ascontiguousarray(a1[:, y0:y0 + HS]),
            "a2py": np.ascontiguousarray(a2py[:, y0:y0 + HP]),
        })
    return in_maps


def _gather_indices():
    """Flat int32 scatter indices: band[jt, dxi, cc, xf] -> cost[b,s,y,x,z]
    (per-core local cost of shape [B, NS, HS, W, D])."""
    if "idx" in _SCHED_CACHE:
        return _SCHED_CACHE["idx"]
    fills, nctot = schedule()
    b_a = np.empty(nctot, np.int64)
    y_a = np.empty(nctot, np.int64)
    z_a = np.empty(nctot, np.int64)
    ys_a = np.empty(nctot, np.int64)
    zs_a = np.empty(nctot, np.int64)
    for f in fills:
        cc0, ny, nz = f["cc0"], f["ny"], f["nz"]
        n = ny * nz
        zz = np.repeat(np.arange(f["z0"], f["z0"] + nz), ny)
        yy = np.tile(np.arange(f["y0"], f["y0"] + ny), nz)
        b_a[cc0:cc0 + n] = f["b"]
        y_a[cc0:cc0 + n] = yy
        z_a[cc0:cc0 + n] = zz
        ys_a[cc0:cc0 + n] = f["ys"]
        zs_a[cc0:cc0 + n] = f["zs"]
    jt = np.arange(NT)[:, None, None, None]
    dxi = np.arange(NSX)[None, :, None, None]
    cc = slice(None)
    xf = np.arange(T)[None, None, None, :]
    bb = b_a[None, None, :, None]
    yy = y_a[None, None, :, None]
    zz = z_a[None, None, :, None]
    s = ((zs_a - z_a + 1)[None, None, :, None] * NSX2
         + (ys_a - y_a)[None, None, :, None] * NSX + dxi)
    x = T * jt + xf
    flat = (((bb * NS + s) * HS + yy) * W + x) * D + zz
    idx = np.ascontiguousarray(flat.astype(np.int32).ravel())
    _SCHED_CACHE["idx"] = idx
    return idx


def _gather_core(gsup_core):
    """gsup [128, nctot*T] bf16 -> band [NT, 9, nctot, T] f32 via one strided
    view copy."""
    fills, nctot = schedule()
    a = np.ascontiguousarray(gsup_core)
    totc = nctot * T
    es = a.dtype.itemsize
    band = np.lib.stride_tricks.as_strided(
        a, shape=(NT, NSX, nctot, T),
        strides=(MT * totc * es, totc * es, T * es, totc * es + es))
    return band.astype(np.float32)


_PROGRAM_CACHE = {}


def kernel(mpi1_features, mpi1_alpha, mpi2_features, mpi2_alpha,
           _trace=False, _trace_kwargs=None):
    if "nc" not in _PROGRAM_CACHE:
        _PROGRAM_CACHE["nc"] = build_program()
    nc = _PROGRAM_CACHE["nc"]
    in_maps = make_in_maps(*prepare_host_inputs(
        mpi1_features, mpi1_alpha, mpi2_features, mpi2_alpha))
    res = run_bass_kernel_spmd(nc, in_maps, list(range(NCORES)),
                               trace=_trace, **(_trace_kwargs or {}))
    idx = _gather_indices()
    costs = []
    cmasks = []
    for k in range(NCORES):
        band = _gather_core(res.results[k]["gsup"])
        c = np.zeros(B * NS * HS * W * D, np.float32)
        c[idx] = band.ravel()
        costs.append(c.reshape(B, NS, HS, W, D))
        sc = np.asarray(res.results[k]["cm_sc"], np.float32)
        cm = np.empty((B, NS, HS, W, D), np.float32)
        for st in range(NST):
            dyi, rem = divmod(st, 5 * NSD)
            dxp, dzi = divmod(rem, NSD)
            dx0 = 2 * dxp
            s0 = dzi * NSX2 + dyi * NSX + dx0
            nds = 2 if dx0 + 1 < NSX else 1
            for dd in range(nds):
                cm[:, s0 + dd] = sc[st, dd * B * HS:(dd + 1) * B * HS
                                    ].reshape(B, HS, W, D)
        cmasks.append(cm)
    cost = np.concatenate(costs, axis=2)
    cmask = np.concatenate(cmasks, axis=2)
    if _trace:
        kernel.last_results = res
    return cost, cmask
